# revision 13
# baseline (speedup 1.0000x reference)
"""Trainium2 Bass kernel for nn_DGCRM_88227218194820.

The reference module's dynamic-adjacency branch (gconv_hyper / nodevec /
adp) is dead code w.r.t. the returned hidden state: due to the faithful
source bug, gconv_rnn(inp, i) == concat([inp, a*inp, a*inp], -1) @ rnn_W[i]
+ rnn_b[i] uses no adjacency, and the normalized adjacencies are deleted.
The output therefore reduces to a per-row GRU gate:

    combined = concat(x, h)                      # [.., 66]
    z  = sigmoid(combined @ Wz + bz)
    r  = sigmoid(combined @ Wr + br)
    hc = tanh(concat(x, r*h) @ Wc + bc)
    out = z*h + (1-z)*hc

with Wg folded from rnn_W: Wg = W[:66] + a*(W[66:132] + W[132:198]),
summed over the two gconv_rnn calls per gate.

Layout (per core, data-parallel over batch: 2 of 16 batches per core,
R = 2048 rows): everything lives transposed (channels on partitions) and
"group-stacked" -- rows 0:1024 (group A) on partitions 0:64, rows
1024:2048 (group B) on partitions 64:128, so every ACT/DVE op uses all
128 partitions.  Each gate matmul uses a K=128 block-diagonal bf16
weight blockdiag(Wg_h, Wg_h), which computes both groups' pre-acts in
one instruction with PSUM output already group-stacked; the 2-channel x
contribution AND the gate bias (as a constant-1 input channel)
accumulate via a K=6 block-diagonal matmul.

dtypes: matmul inputs bf16 (fp32 PE matmul is ~4x slower), PSUM
accumulation fp32, activations + gating arithmetic bf16 (fp32
tensor_tensor on the DVE has no fast mode; bf16 runs 2x), output bf16
(upcast on host).  Measured end-to-end relative error ~4e-3.

Perf structure:
 - a PE warm-up burst of dummy matmuls runs while the input DMAs are in
   flight, so the HAM clock gate reaches 2.4 GHz before the real matmuls
 - input DMAs are merged (bitcast-packed) and ordered so the first
   matmul's operands land first (htb split in halves)
 - instruction emission is flattened so the ACT engine (the mid-kernel
   bottleneck: 6 serial activations) never waits on PE/DVE
 - the last block's gating chain runs at 256-col granularity so the
   final output DMA (whose ~2us HBM write receipt gates the exit
   barrier) issues as early as possible
"""

import ml_dtypes
import numpy as np

import concourse.tile as tile
from concourse import bacc, mybir
from concourse.bass_utils import run_bass_kernel_spmd

N_CORES = 8
B, N, IN_DIM, HID = 16, 1024, 2, 64
GC_ALPHA = 0.05
CIN = HID + IN_DIM          # 66
R = (B // N_CORES) * N      # 2048 rows per core
G = R // 2                  # 1024 rows per group (A/B)
BLK = 512                   # psum free-dim block
NBLK = G // BLK             # 2
N_WARMUP_MM = 6

F32 = mybir.dt.float32
BF16 = mybir.dt.bfloat16
AF = mybir.ActivationFunctionType
BF16_NP = ml_dtypes.bfloat16

_program_cache = {}


def build_program():
    # Bacc (not raw Bass): its compile() runs move_matmul_waits_to_ldweights
    # + generate_event_semaphores, which split multi-sem waits to satisfy
    # the TRN2 "at most 1 sync wait per instruction" constraint.
    nc = bacc.Bacc()
    htb = nc.dram_tensor("htb", [128, G], BF16, kind="ExternalInput")
    # aux1: bf16 blockdiag gate weights, bitcast-packed as f32
    aux1 = nc.dram_tensor("aux1", [128, 192], F32, kind="ExternalInput")
    # aux2: bf16 blockdiag x+bias weights and x+ones data, bitcast-packed
    aux2 = nc.dram_tensor("aux2", [6, 704], F32, kind="ExternalInput")
    ot = nc.dram_tensor("ot", [128, G], BF16, kind="ExternalOutput")

    with tile.TileContext(nc) as tc:
        with (
            tc.tile_pool(name="sb", bufs=1) as sb,
            tc.tile_pool(name="ps", bufs=1, space="PSUM") as ps,
        ):
            HTB = sb.tile([128, G], BF16, tag="HTB")
            AUX1 = sb.tile([128, 192], F32, tag="AUX1")
            AUX2 = sb.tile([6, 704], F32, tag="AUX2")
            ZT = sb.tile([128, G], BF16, tag="ZT")
            RT = sb.tile([128, G], BF16, tag="RT")
            RHB = sb.tile([128, G], BF16, tag="RHB")
            HC = sb.tile([128, G], BF16, tag="HC")
            DD = sb.tile([128, G], BF16, tag="DD")
            ZD = sb.tile([128, G], BF16, tag="ZD")
            OT = sb.tile([128, G], BF16, tag="OT")
            WARM = sb.tile([128, BLK], BF16, tag="WARM")
            dummy = sb.tile([1, 1], F32, tag="dummy")

            WB = AUX1[:, 0:192].bitcast(BF16)    # [128, 384]
            WX = AUX2[:, 0:192].bitcast(BF16)    # [6, 384]
            XT = AUX2[:, 192:704].bitcast(BF16)  # [6, 1024]

            # Fire the ACT table load (sigmoid_and_others, covers tanh)
            # immediately so it overlaps the input DMAs.  Use an AP bias
            # to avoid constant-tensor init boilerplate.
            nc.vector.memset(dummy, 0.0)
            nc.scalar.activation(
                out=dummy, in_=dummy, func=AF.Sigmoid, bias=dummy[0:1, 0:1]
            )

            # Input DMAs in need-order (each 2D dma_start costs ~0.6us of
            # HWDGE descriptor generation on SP, serial per engine).
            nc.sync.dma_start(out=HTB[:, 0:BLK], in_=htb[:, 0:BLK])
            nc.sync.dma_start(out=AUX1, in_=aux1[:, :])
            nc.sync.dma_start(out=AUX2, in_=aux2[:, :])
            nc.sync.dma_start(out=HTB[:, BLK:G], in_=htb[:, BLK:G])

            # PE warm-up: dummy matmuls while DMAs are in flight keep the
            # HAM activity window busy so real matmuls run at 2.4 GHz.
            nc.vector.memset(WARM, 0.0)
            pwarm = ps.tile([128, BLK], F32, tag="pwarm")
            for _ in range(N_WARMUP_MM):
                nc.tensor.matmul(
                    pwarm[:, :], WARM[:, 0:128], WARM[:, :],
                    start=True, stop=True, skip_group_check=True,
                )

            def mm_h(psum_t, g, rhs_t, cols, n=BLK):
                wc = slice(128 * g, 128 * g + 128)
                nc.tensor.matmul(
                    psum_t[:, 0:n], WB[:, wc], rhs_t[:, cols],
                    start=False, stop=True, skip_group_check=True,
                )

            def mm_xb(psum_t, g, cols, n=BLK):
                # x channels + constant-1 bias channel, K=6 blockdiag
                wc = slice(128 * g, 128 * g + 128)
                nc.tensor.matmul(
                    psum_t[:, 0:n], WX[0:6, wc], XT[0:6, cols],
                    start=True, stop=False, skip_group_check=True,
                )

            cols0 = slice(0, BLK)
            cols1 = slice(BLK, G)
            pr0 = ps.tile([128, BLK], F32, tag="pr0")
            pz0 = ps.tile([128, BLK], F32, tag="pz0")
            pr1 = ps.tile([128, BLK], F32, tag="pr1")
            pz1 = ps.tile([128, BLK], F32, tag="pz1")
            pc0 = ps.tile([128, BLK], F32, tag="pc0")
            pc1 = ps.tile([128, BLK], F32, tag="pc1")

            # ---- flattened schedule (PE FIFO / ACT FIFO tuned) ----
            # PE: r0, z0, r1, z1, c0, c1   ACT: r0, z0, r1, tanh0, z1, tanh1
            mm_xb(pr0, 1, cols0)
            mm_h(pr0, 1, HTB, cols0)
            mm_xb(pz0, 0, cols0)
            mm_h(pz0, 0, HTB, cols0)
            nc.scalar.activation(out=RT[:, cols0], in_=pr0[:, :], func=AF.Sigmoid)
            nc.vector.tensor_mul(RHB[:, cols0], RT[:, cols0], HTB[:, cols0])

            mm_xb(pr1, 1, cols1)
            mm_h(pr1, 1, HTB, cols1)
            mm_xb(pz1, 0, cols1)
            mm_h(pz1, 0, HTB, cols1)
            nc.scalar.activation(out=ZT[:, cols0], in_=pz0[:, :], func=AF.Sigmoid)

            mm_xb(pc0, 2, cols0)
            mm_h(pc0, 2, RHB, cols0)
            nc.scalar.activation(out=RT[:, cols1], in_=pr1[:, :], func=AF.Sigmoid)
            nc.vector.tensor_mul(RHB[:, cols1], RT[:, cols1], HTB[:, cols1])
            nc.scalar.activation(out=HC[:, cols0], in_=pc0[:, :], func=AF.Tanh)

            mm_xb(pc1, 2, cols1)
            mm_h(pc1, 2, RHB, cols1)
            nc.scalar.activation(out=ZT[:, cols1], in_=pz1[:, :], func=AF.Sigmoid)
            nc.scalar.activation(out=HC[:, cols1], in_=pc1[:, :], func=AF.Tanh)

            # gating chain, block 0 at full width
            nc.vector.tensor_sub(DD[:, cols0], HTB[:, cols0], HC[:, cols0])
            nc.vector.tensor_mul(ZD[:, cols0], ZT[:, cols0], DD[:, cols0])
            nc.vector.tensor_add(OT[:, cols0], HC[:, cols0], ZD[:, cols0])
            nc.sync.dma_start(out=ot[:, cols0], in_=OT[:, cols0])

            # last block at 256-col granularity: the final DMA's ~2us HBM
            # write receipt gates the exit barrier, so issue it early.
            half = BLK // 2
            for q in range(2):
                c = slice(BLK + q * half, BLK + (q + 1) * half)
                nc.vector.tensor_sub(DD[:, c], HTB[:, c], HC[:, c])
                nc.vector.tensor_mul(ZD[:, c], ZT[:, c], DD[:, c])
                nc.vector.tensor_add(OT[:, c], HC[:, c], ZD[:, c])
                nc.sync.dma_start(out=ot[:, c], in_=OT[:, c])

    nc.compile()
    return nc


def get_program():
    if "nc" not in _program_cache:
        _program_cache["nc"] = build_program()
    return _program_cache["nc"]


def fold_params(rnn_W, rnn_b):
    """Fold the gconv_rnn bug + gate sums into per-gate [66,64] weights."""
    Wf = rnn_W[:, :CIN, :] + GC_ALPHA * (
        rnn_W[:, CIN : 2 * CIN, :] + rnn_W[:, 2 * CIN : 3 * CIN, :]
    )  # [6, 66, 64]
    Wg = np.stack([Wf[0] + Wf[1], Wf[2] + Wf[3], Wf[4] + Wf[5]])  # [3,66,64]
    bg = np.stack(
        [rnn_b[0] + rnn_b[1], rnn_b[2] + rnn_b[3], rnn_b[4] + rnn_b[5]]
    )  # [3, 64]
    return Wg, bg


def make_in_maps(x, h, rnn_W, rnn_b):
    Wg, bg = fold_params(rnn_W, rnn_b)
    # combined = concat(x, h): channels 0:2 are x, 2:66 are h.
    # Gate order in the packed weights: z=0, r=1, c=2.
    W_x = Wg[:, :IN_DIM, :]  # [3, 2, 64]
    W_h = Wg[:, IN_DIM:, :]  # [3, 64, 64]

    # Block-diagonal bf16 weights: gate g occupies cols 128g:128(g+1);
    # out = blockdiag(Wg_h, Wg_h).T @ [h_A; h_B] = [gate_A; gate_B].
    # wx rows per group: [x0; x1; 1] -> [Wg_x; bg] folds the bias in.
    wb_host = np.zeros((128, 384), BF16_NP)
    wx_host = np.zeros((6, 384), BF16_NP)
    for g in range(3):
        wb_host[0:64, 128 * g : 128 * g + 64] = W_h[g]
        wb_host[64:128, 128 * g + 64 : 128 * g + 128] = W_h[g]
        wx_host[0:2, 128 * g : 128 * g + 64] = W_x[g]
        wx_host[2, 128 * g : 128 * g + 64] = bg[g]
        wx_host[3:5, 128 * g + 64 : 128 * g + 128] = W_x[g]
        wx_host[5, 128 * g + 64 : 128 * g + 128] = bg[g]

    aux1_host = np.ascontiguousarray(wb_host.view(np.float32))  # [128, 192]

    hf = h.reshape(N_CORES, R, HID)
    xf = x.reshape(N_CORES, R, IN_DIM)
    in_maps = []
    for c in range(N_CORES):
        ht_host = np.ascontiguousarray(
            np.concatenate([hf[c, :G].T, hf[c, G:].T], axis=0)
        )  # [128, G] f32
        xt_host = np.empty((6, G), BF16_NP)
        xt_host[0:2] = xf[c, :G].T
        xt_host[2] = 1.0
        xt_host[3:5] = xf[c, G:].T
        xt_host[5] = 1.0
        aux2_host = np.empty((6, 704), np.float32)
        aux2_host[:, 0:192] = wx_host.view(np.float32)
        aux2_host[:, 192:704] = xt_host.view(np.float32)
        in_maps.append(
            dict(
                htb=ht_host.astype(BF16_NP),
                aux1=aux1_host,
                aux2=aux2_host,
            )
        )
    return in_maps


def gather_output(results):
    outs = []
    for c in range(N_CORES):
        o = np.asarray(results[c]["ot"]).astype(np.float32)  # [128, G]
        outs.append(np.concatenate([o[:64].T, o[64:].T], axis=0))  # [R, HID]
    return (
        np.concatenate(outs, axis=0).reshape(B, N, HID).astype(np.float32)
    )


def run(inputs, trace=False, **kw):
    x = np.ascontiguousarray(np.asarray(inputs["x"], dtype=np.float32))
    h = np.ascontiguousarray(
        np.asarray(inputs["hidden_state"], dtype=np.float32)
    )
    rnn_W = np.asarray(inputs["rnn_W"], dtype=np.float32)
    rnn_b = np.asarray(inputs["rnn_b"], dtype=np.float32)

    in_maps = make_in_maps(x, h, rnn_W, rnn_b)
    nc = get_program()
    res = run_bass_kernel_spmd(
        nc, in_maps, core_ids=list(range(N_CORES)), trace=trace, **kw
    )
    return gather_output(res.results), res


def kernel(**inputs) -> np.ndarray:
    out, _ = run(inputs)
    return out
